# revision 1
# baseline (speedup 1.0000x reference)
"""Sparse-attention (sliding window 512 + front 256) Trainium2 kernel.

Head-sharded across 8 NeuronCores: core c computes q-heads {2c, 2c+1} and
kv-head c//2, producing a partial output y_c = attn_out_c @ wo_c; the host
sums the 8 partials.

Layout choices:
  - q/k projections are computed transposed (qT: [dqk, seq]) directly from a
    host-provided x^T, so score matmuls need no transposes.
  - RoPE uses a "paired" head-dim basis (reals in dims 0..63, imags 64..127),
    obtained by permuting wq/wk columns on the host. Dot products are
    permutation-invariant, so scores are unchanged.
  - Sparse mask: per 128x128 tile the mask is full, causal (b<=a) or
    anti-causal (b>a); only 2 nontrivial patterns, passed as constants.
  - Softmax without max-subtraction (scores ~ N(0,1) after 1/sqrt(128) scale,
    so exp() is safe in fp32), sum fused into the exp via accum_out.
"""

import math
import sys

import numpy as np

sys.path.insert(0, "/opt/trn_rl_repo")

import concourse.bass as bass
from concourse import bacc
import concourse.mybir as mybir
import concourse.tile as tile
from concourse.bass_utils import run_bass_kernel_spmd

# Problem constants (hardcoded per contract)
S = 4096
D = 2048
NH = 16
NKV = 4
DQK = 128
DV = 128
WIN = 512
FRONT = 256
THETA = 10000.0
P = 128
NQT = S // P  # 32 query tiles
NC_ = 8  # cores
SC = 512  # seq chunk for projections
NSC = S // SC  # 8
KO = D // P  # 16 contraction chunks

F32 = mybir.dt.float32
BF16 = mybir.dt.bfloat16

NEG = -1.0e9


def _key_tiles(qt):
    """Key tiles for query tile qt: list of (kt, mask) with mask in
    {'full','causal','anti'}; tiles are contiguous groups for matmul."""
    if qt <= 5:
        tiles = [(kt, "full") for kt in range(qt)] + [(qt, "causal")]
    else:
        tiles = [(0, "full"), (1, "full"), (qt - 4, "anti")]
        tiles += [(kt, "full") for kt in range(qt - 3, qt)]
        tiles += [(qt, "causal")]
    return tiles


def build_program():
    nc = bacc.Bacc(None, target_bir_lowering=False)

    xt = nc.dram_tensor("xt", [D, S], BF16, kind="ExternalInput")
    wq_d = nc.dram_tensor("wq", [D, 2 * DQK], BF16, kind="ExternalInput")
    wk_d = nc.dram_tensor("wk", [D, DQK], BF16, kind="ExternalInput")
    wv_d = nc.dram_tensor("wv", [D, DV], BF16, kind="ExternalInput")
    wo_d = nc.dram_tensor("wo", [2 * DV, D], BF16, kind="ExternalInput")
    cos_d = nc.dram_tensor("cosd", [P, S], F32, kind="ExternalInput")
    sin_d = nc.dram_tensor("sind", [P, S], F32, kind="ExternalInput")
    mask_c_d = nc.dram_tensor("maskc", [P, P], F32, kind="ExternalInput")
    mask_a_d = nc.dram_tensor("maska", [P, P], F32, kind="ExternalInput")
    ident_d = nc.dram_tensor("ident", [P, P], BF16, kind="ExternalInput")
    y_d = nc.dram_tensor("y", [S, D], F32, kind="ExternalOutput")

    inv_sqrt_dqk = 1.0 / math.sqrt(DQK)

    with tile.TileContext(nc) as tc:
        with (
            tc.tile_pool(name="persist", bufs=1) as persist,
            tc.tile_pool(name="xchunk", bufs=2) as xpool,
            tc.tile_pool(name="stage", bufs=3) as stage,
            tc.tile_pool(name="ppool", bufs=3) as ppool,
            tc.tile_pool(name="lpool", bufs=4) as lpool,
            tc.tile_pool(name="ptpool", bufs=3) as ptpool,
            tc.tile_pool(name="ystage", bufs=2) as ypool,
            tc.tile_pool(name="psA", bufs=2, space="PSUM") as psA,
            tc.tile_pool(name="psS", bufs=2, space="PSUM") as psS,
            tc.tile_pool(name="psT", bufs=1, space="PSUM") as psT,
            tc.tile_pool(name="psO", bufs=1, space="PSUM") as psO,
        ):
            # ---- persistent SBUF tensors ----
            qT = persist.tile([P, 2, S], BF16, tag="qT")
            kT = persist.tile([P, S], BF16, tag="kT")
            v_sb = persist.tile([P, NQT, DV], BF16, tag="v")
            outT = persist.tile([P, 2, NQT, P], BF16, tag="outT")
            cos_sb = persist.tile([P, S], F32, tag="cos")
            sin_sb = persist.tile([P, S], F32, tag="sin")
            wq_sb = persist.tile([P, KO, 2 * DQK], BF16, tag="wq")
            wk_sb = persist.tile([P, KO, DQK], BF16, tag="wk")
            wv_sb = persist.tile([P, KO, DV], BF16, tag="wv")
            wo_sb = persist.tile([P, 2, D], BF16, tag="wo")
            mask_c = persist.tile([P, P], F32, tag="mc")
            mask_a = persist.tile([P, P], F32, tag="ma")
            ident = persist.tile([P, P], BF16, tag="id")

            nc.sync.dma_start(cos_sb[:], cos_d[:])
            nc.sync.dma_start(sin_sb[:], sin_d[:])
            nc.sync.dma_start(wq_sb[:], wq_d.rearrange("(ko p) m -> p ko m", p=P))
            nc.sync.dma_start(wk_sb[:], wk_d.rearrange("(ko p) m -> p ko m", p=P))
            nc.sync.dma_start(wv_sb[:], wv_d.rearrange("(ko p) m -> p ko m", p=P))
            nc.sync.dma_start(wo_sb[:], wo_d.rearrange("(h p) n -> p h n", p=P))
            nc.sync.dma_start(mask_c[:], mask_c_d[:])
            nc.sync.dma_start(mask_a[:], mask_a_d[:])
            nc.sync.dma_start(ident[:], ident_d[:])

            xt_r = xt.rearrange("(ko p) s -> p ko s", p=P)

            # ---- Phase A: projections + RoPE ----
            for sc in range(NSC):
                ssl = slice(sc * SC, (sc + 1) * SC)
                xch = xpool.tile([P, KO, SC], BF16, tag="xch")
                nc.sync.dma_start(xch[:], xt_r[:, :, ssl])

                # qT (2 head tiles) and kT, with RoPE
                for m in range(3):
                    ps = psA.tile([P, SC], F32, tag="psA")
                    if m < 2:
                        w_ap = wq_sb[:, :, m * P : (m + 1) * P]
                        dst = qT[:, m, ssl]
                    else:
                        w_ap = wk_sb
                        dst = kT[:, ssl]
                    for ko in range(KO):
                        nc.tensor.matmul(
                            ps[:],
                            w_ap[:, ko, :],
                            xch[:, ko, :],
                            start=(ko == 0),
                            stop=(ko == KO - 1),
                        )
                    st = stage.tile([P, SC], F32, tag="ropestage")
                    nc.scalar.copy(st[:], ps[:])
                    # RoPE (paired layout): rows 0:64 real, 64:128 imag
                    trc = stage.tile([P, SC], F32, tag="trc")
                    trs = stage.tile([P, SC], F32, tag="trs")
                    nc.vector.tensor_tensor(
                        trc[:], st[:], cos_sb[:, ssl], op=mybir.AluOpType.mult
                    )
                    nc.vector.tensor_tensor(
                        trs[:], st[:], sin_sb[:, ssl], op=mybir.AluOpType.mult
                    )
                    # out_r = r*c - i*s ; out_i = r*s + i*c
                    # (DVE needs equal base partitions for both SBUF inputs,
                    #  so stage the upper halves at partition 0 first)
                    his = stage.tile([64, SC], F32, tag="his")
                    hic = stage.tile([64, SC], F32, tag="hic")
                    nc.scalar.copy(his[:], trs[64:128])
                    nc.scalar.copy(hic[:], trc[64:128])
                    nc.vector.tensor_tensor(
                        dst[0:64], trc[0:64], his[:], op=mybir.AluOpType.subtract
                    )
                    nc.vector.tensor_tensor(
                        dst[64:128], trs[0:64], hic[:], op=mybir.AluOpType.add
                    )

                # v natural layout: [seq, dv] per key tile
                for j in range(SC // P):
                    kt_idx = sc * (SC // P) + j
                    psv = psA.tile([P, SC], F32, tag="psA")
                    for ko in range(KO):
                        nc.tensor.matmul(
                            psv[:, :DV],
                            xch[:, ko, j * P : (j + 1) * P],
                            wv_sb[:, ko, :],
                            start=(ko == 0),
                            stop=(ko == KO - 1),
                        )
                    nc.scalar.copy(v_sb[:, kt_idx, :], psv[:, :DV])

            # ---- Phase B: attention ----
            for h in range(2):
                for qt in range(NQT):
                    tiles = _key_tiles(qt)
                    nk = len(tiles)
                    w = nk * P
                    qh = qT[:, h, qt * P : (qt + 1) * P]

                    ps_s = psS.tile([P, 7 * P], F32, tag="psS", name="ps_s")
                    ps_s = ps_s[:, :w]
                    # contiguous matmul groups, split at psum bank (512) bounds
                    groups = []  # (dst_lo, kt_lo, width)
                    pos = 0
                    i = 0
                    while i < nk:
                        j = i
                        while j + 1 < nk and tiles[j + 1][0] == tiles[j][0] + 1:
                            j += 1
                        glo, gw = tiles[i][0], (j - i + 1) * P
                        # split so no matmul crosses a 512-col psum boundary
                        off = 0
                        while off < gw:
                            room = 512 - ((pos + off) % 512)
                            take = min(gw - off, room, 512)
                            groups.append((pos + off, glo * P + off, take))
                            off += take
                        pos += gw
                        i = j + 1
                    for dst_lo, src_lo, gw in groups:
                        nc.tensor.matmul(
                            ps_s[:, dst_lo : dst_lo + gw],
                            qh,
                            kT[:, src_lo : src_lo + gw],
                            start=True,
                            stop=True,
                        )
                    # masks
                    for idx, (kt, mk) in enumerate(tiles):
                        if mk == "full":
                            continue
                        msk = mask_c if mk == "causal" else mask_a
                        nc.vector.tensor_tensor(
                            ps_s[:, idx * P : (idx + 1) * P],
                            ps_s[:, idx * P : (idx + 1) * P],
                            msk[:],
                            op=mybir.AluOpType.add,
                        )
                    # exp + row-sum
                    p_sb = ppool.tile([P, 7 * P], BF16, tag="p", name="p_sb")
                    p_sb = p_sb[:, :w]
                    lsum = lpool.tile([P, 1], F32, tag="l")
                    nc.scalar.activation(
                        p_sb,
                        ps_s,
                        mybir.ActivationFunctionType.Exp,
                        scale=inv_sqrt_dqk,
                        accum_out=lsum[:],
                    )
                    rl = lpool.tile([P, 1], F32, tag="rl")
                    nc.vector.reciprocal(rl[:], lsum[:])
                    nc.vector.tensor_tensor(
                        p_sb, p_sb, rl.to_broadcast((P, w)), op=mybir.AluOpType.mult
                    )
                    # transpose p tiles; accumulate out
                    ps_o = psO.tile([P, P], F32, tag="psO")
                    for idx, (kt, mk) in enumerate(tiles):
                        ps_t = psT.tile([P, P], BF16, tag="psT")
                        nc.tensor.transpose(
                            ps_t[:], p_sb[:, idx * P : (idx + 1) * P], ident[:]
                        )
                        pt_sb = ptpool.tile([P, P], BF16, tag="pt")
                        nc.scalar.copy(pt_sb[:], ps_t[:])
                        nc.tensor.matmul(
                            ps_o[:],
                            v_sb[:, kt, :],
                            pt_sb[:],
                            start=(idx == 0),
                            stop=(idx == nk - 1),
                        )
                    nc.scalar.copy(outT[:, h, qt, :], ps_o[:])

            # ---- Phase C: y = outT.T @ wo ----
            for st_i in range(NQT):
                ys = ypool.tile([P, D], F32, tag="y")
                for nn in range(4):
                    ps_y = psA.tile([P, SC], F32, tag="psA")
                    for h in range(2):
                        nc.tensor.matmul(
                            ps_y[:],
                            outT[:, h, st_i, :],
                            wo_sb[:, h, nn * SC : (nn + 1) * SC],
                            start=(h == 0),
                            stop=(h == 1),
                        )
                    nc.scalar.copy(ys[:, nn * SC : (nn + 1) * SC], ps_y[:])
                nc.sync.dma_start(y_d[st_i * P : (st_i + 1) * P, :], ys[:])

    return nc


_PROGRAM = None


def _get_program():
    global _PROGRAM
    if _PROGRAM is None:
        _PROGRAM = build_program()
        # Bacc legalization (register alloc, 1-wait-per-instruction split)
        # must run before serialization; the prebuilt-nc PJRT path does not
        # call finalize itself.
        _PROGRAM.finalize()
    return _PROGRAM


def _host_inputs(x, wq, wk, wv, wo):
    """Build per-core input maps (host-side sharding + preprocessing)."""
    x2 = np.asarray(x).reshape(S, D).astype(np.float32)
    xt_bf = x2.T.astype(ml_bf16)

    # paired RoPE basis permutation within each head
    perm = np.concatenate([np.arange(0, DQK, 2), np.arange(1, DQK, 2)])
    wq_p = np.asarray(wq).reshape(D, NH, DQK)[:, :, perm]
    wk_p = np.asarray(wk).reshape(D, NKV, DQK)[:, :, perm]
    wv_r = np.asarray(wv).reshape(D, NKV, DV)
    wo_r = np.asarray(wo).reshape(NH, DV, D)

    inv_freq = 1.0 / (THETA ** (np.arange(0, DQK, 2)[: DQK // 2] / DQK))
    t = np.arange(S, dtype=np.float64)
    ang = np.outer(t, inv_freq)  # (S, 64)
    cos_half = np.cos(ang).T.astype(np.float32)  # (64, S)
    sin_half = np.sin(ang).T.astype(np.float32)
    cos_dup = np.concatenate([cos_half, cos_half], 0)  # (128, S)
    sin_dup = np.concatenate([sin_half, sin_half], 0)

    a = np.arange(P)[:, None]
    b = np.arange(P)[None, :]
    mask_c = np.where(b <= a, 0.0, NEG).astype(np.float32)
    mask_a = np.where(b > a, 0.0, NEG).astype(np.float32)
    ident = np.eye(P, dtype=np.float32).astype(ml_bf16)

    in_maps = []
    for c in range(NC_):
        kvh = c // 2
        in_maps.append(
            {
                "xt": xt_bf,
                "wq": np.ascontiguousarray(
                    wq_p[:, 2 * c : 2 * c + 2, :].reshape(D, 2 * DQK)
                ).astype(ml_bf16),
                "wk": np.ascontiguousarray(wk_p[:, kvh, :]).astype(ml_bf16),
                "wv": np.ascontiguousarray(wv_r[:, kvh, :]).astype(ml_bf16),
                "wo": np.ascontiguousarray(
                    wo_r[2 * c : 2 * c + 2].reshape(2 * DV, D)
                ).astype(ml_bf16),
                "cosd": cos_dup,
                "sind": sin_dup,
                "maskc": mask_c,
                "maska": mask_a,
                "ident": ident,
            }
        )
    return in_maps


try:
    import ml_dtypes

    ml_bf16 = ml_dtypes.bfloat16
except ImportError:  # pragma: no cover
    ml_bf16 = np.float32


def kernel(x, wq, wk, wv, wo, _trace=False, _trace_kwargs=None):
    nc = _get_program()
    in_maps = _host_inputs(x, wq, wk, wv, wo)
    res = run_bass_kernel_spmd(
        nc, in_maps, list(range(NC_)), trace=_trace, **(_trace_kwargs or {})
    )
    y = np.zeros((S, D), np.float32)
    for r in res.results:
        y += np.asarray(r["y"], np.float32)
    out = y.reshape(1, S, D)
    if _trace:
        return out, res
    return out



# revision 5
# speedup vs baseline: 1.3193x; 1.3193x over previous
"""Sparse-attention (sliding window 512 + front 256) Trainium2 kernel.

Head-sharded across 8 NeuronCores: core c computes q-heads {2c, 2c+1} and
kv-head c//2, producing a partial output y_c = attn_out_c @ wo_c; the host
sums the 8 partials.

Layout choices:
  - q/k projections are computed transposed (qT: [dqk, seq]) directly from a
    host-provided x^T, so score matmuls need no transposes.
  - RoPE uses a "paired" head-dim basis (reals in dims 0..63, imags 64..127),
    obtained by permuting wq/wk columns on the host. Dot products are
    permutation-invariant, so scores are unchanged.
  - Scores are computed TRANSPOSED (st[k, q] via lhsT=k-tile, rhs=q-tile), so
    the PV matmul (lhsT=v-tile, rhs=exp-tile) directly yields outT [dv, q]
    with no 128x128 transposes of the probability matrix.
  - Both q-heads of the core share the kv head, so exp/l/PV run on packed
    [k, 2*128] tiles (h0|h1 side by side).
  - Softmax denominators l[q] = sum_k exp come from a ones-column matmul
    (lhsT=[128,1] ones) accumulated over key tiles; 1/l is broadcast to all
    partitions with a rank-1 outer-product matmul (lhsT=[1,128] ones).
  - Sparse mask: per 128x128 tile the mask is full, causal or anti-causal;
    the two nontrivial patterns are passed as constants (transposed, and
    duplicated for the packed two-head layout).
  - Softmax without max-subtraction (scores ~ N(0,1) after 1/sqrt(128) scale,
    so exp() is safe in fp32).
"""

import math
import sys

import numpy as np

sys.path.insert(0, "/opt/trn_rl_repo")

import concourse.bass as bass
from concourse import bacc
import concourse.mybir as mybir
import concourse.tile as tile
from concourse.bass_utils import run_bass_kernel_spmd

# Problem constants (hardcoded per contract)
S = 4096
D = 2048
NH = 16
NKV = 4
DQK = 128
DV = 128
WIN = 512
FRONT = 256
THETA = 10000.0
P = 128
NQT = S // P  # 32 query tiles
NC_ = 8  # cores
SC = 512  # seq chunk for projections
NSC = S // SC  # 8
KO = D // P  # 16 contraction chunks

F32 = mybir.dt.float32
BF16 = mybir.dt.bfloat16

NEG = -1.0e9


def _key_tiles(qt):
    """Key tiles for query tile qt: list of (kt, mask) with mask in
    {'full','causal','anti'}."""
    if qt <= 5:
        tiles = [(kt, "full") for kt in range(qt)] + [(qt, "causal")]
    else:
        tiles = [(0, "full"), (1, "full"), (qt - 4, "anti")]
        tiles += [(kt, "full") for kt in range(qt - 3, qt)]
        tiles += [(qt, "causal")]
    return tiles


def build_program():
    nc = bacc.Bacc(None, target_bir_lowering=False)

    xt = nc.dram_tensor("xt", [D, S], BF16, kind="ExternalInput")
    wq_d = nc.dram_tensor("wq", [D, 2 * DQK], BF16, kind="ExternalInput")
    wk_d = nc.dram_tensor("wk", [D, DQK], BF16, kind="ExternalInput")
    wv_d = nc.dram_tensor("wv", [D, DV], BF16, kind="ExternalInput")
    wo_d = nc.dram_tensor("wo", [2 * DV, D], BF16, kind="ExternalInput")
    cos_d = nc.dram_tensor("cosd", [P, S], F32, kind="ExternalInput")
    sin_d = nc.dram_tensor("sind", [P, S], F32, kind="ExternalInput")
    mask_c_d = nc.dram_tensor("maskcT", [P, 2 * P], F32, kind="ExternalInput")
    mask_a_d = nc.dram_tensor("maskaT", [P, 2 * P], F32, kind="ExternalInput")
    onesc_d = nc.dram_tensor("onesc", [P, 1], BF16, kind="ExternalInput")
    ones1_d = nc.dram_tensor("ones1", [1, P], F32, kind="ExternalInput")
    y_d = nc.dram_tensor("y", [S, D], F32, kind="ExternalOutput")

    inv_sqrt_dqk = 1.0 / math.sqrt(DQK)

    with tile.TileContext(nc) as tc:
        with (
            tc.tile_pool(name="persist", bufs=1) as persist,
            tc.tile_pool(name="xchunk", bufs=2) as xpool,
            tc.tile_pool(name="stage", bufs=3) as stage,
            tc.tile_pool(name="epool", bufs=2) as epool,
            tc.tile_pool(name="lpool", bufs=2) as lpool,
            tc.tile_pool(name="rbpool", bufs=2) as rbpool,
            tc.tile_pool(name="ystage", bufs=2) as ypool,
        ):
            # ---- persistent SBUF tensors ----
            qT = persist.tile([P, 2, S], BF16, tag="qT")
            kT = persist.tile([P, S], BF16, tag="kT")
            v_sb = persist.tile([P, NQT, DV], BF16, tag="v")
            outT = persist.tile([P, 2, NQT, P], BF16, tag="outT")
            cos_sb = persist.tile([P, S], F32, tag="cos")
            sin_sb = persist.tile([P, S], F32, tag="sin")
            wq_sb = persist.tile([P, KO, 2 * DQK], BF16, tag="wq")
            wk_sb = persist.tile([P, KO, DQK], BF16, tag="wk")
            wv_sb = persist.tile([P, KO, DV], BF16, tag="wv")
            wo_sb = persist.tile([P, 2, D], BF16, tag="wo")
            mask_c = persist.tile([P, 2, P], F32, tag="mc")
            mask_a = persist.tile([P, 2, P], F32, tag="ma")
            onesc = persist.tile([P, 1], BF16, tag="onesc")
            ones1 = persist.tile([1, P], F32, tag="ones1")

            nc.sync.dma_start(cos_sb[:], cos_d[:])
            nc.sync.dma_start(sin_sb[:], sin_d[:])
            nc.sync.dma_start(wq_sb[:], wq_d.rearrange("(ko p) m -> p ko m", p=P))
            nc.sync.dma_start(wk_sb[:], wk_d.rearrange("(ko p) m -> p ko m", p=P))
            nc.sync.dma_start(wv_sb[:], wv_d.rearrange("(ko p) m -> p ko m", p=P))
            nc.sync.dma_start(wo_sb[:], wo_d.rearrange("(h p) n -> p h n", p=P))
            nc.sync.dma_start(
                mask_c[:], mask_c_d.rearrange("p (h q) -> p h q", h=2)
            )
            nc.sync.dma_start(
                mask_a[:], mask_a_d.rearrange("p (h q) -> p h q", h=2)
            )
            nc.sync.dma_start(onesc[:], onesc_d[:])
            nc.sync.dma_start(ones1[:], ones1_d[:])

            xt_r = xt.rearrange("(ko p) s -> p ko s", p=P)

            # ---- Phase A: projections + RoPE ----
            with tc.tile_pool(name="psA", bufs=2, space="PSUM") as psA:
                for sc in range(NSC):
                    ssl = slice(sc * SC, (sc + 1) * SC)
                    xch = xpool.tile([P, KO, SC], BF16, tag="xch")
                    nc.sync.dma_start(xch[:], xt_r[:, :, ssl])

                    # qT (2 head tiles) and kT, with RoPE
                    for m in range(3):
                        ps = psA.tile([P, SC], F32, tag="psA")
                        if m < 2:
                            w_ap = wq_sb[:, :, m * P : (m + 1) * P]
                            dst = qT[:, m, ssl]
                        else:
                            w_ap = wk_sb
                            dst = kT[:, ssl]
                        for ko in range(KO):
                            nc.tensor.matmul(
                                ps[:],
                                w_ap[:, ko, :],
                                xch[:, ko, :],
                                start=(ko == 0),
                                stop=(ko == KO - 1),
                            )
                        st = stage.tile([P, SC], F32, tag="ropestage")
                        nc.scalar.copy(st[:], ps[:])
                        # RoPE (paired layout): rows 0:64 real, 64:128 imag
                        trc = stage.tile([P, SC], F32, tag="trc")
                        trs = stage.tile([P, SC], F32, tag="trs")
                        nc.vector.tensor_tensor(
                            trc[:], st[:], cos_sb[:, ssl], op=mybir.AluOpType.mult
                        )
                        nc.vector.tensor_tensor(
                            trs[:], st[:], sin_sb[:, ssl], op=mybir.AluOpType.mult
                        )
                        # out_r = r*c - i*s ; out_i = r*s + i*c
                        # (DVE needs equal base partitions for both SBUF inputs,
                        #  so stage the upper halves at partition 0 first)
                        his = stage.tile([64, SC], F32, tag="his")
                        hic = stage.tile([64, SC], F32, tag="hic")
                        nc.scalar.copy(his[:], trs[64:128])
                        nc.scalar.copy(hic[:], trc[64:128])
                        nc.vector.tensor_tensor(
                            dst[0:64], trc[0:64], his[:], op=mybir.AluOpType.subtract
                        )
                        nc.vector.tensor_tensor(
                            dst[64:128], trs[0:64], hic[:], op=mybir.AluOpType.add
                        )

                    # v natural layout: [seq, dv] per key tile
                    for j in range(SC // P):
                        kt_idx = sc * (SC // P) + j
                        psv = psA.tile([P, SC], F32, tag="psA")
                        for ko in range(KO):
                            nc.tensor.matmul(
                                psv[:, :DV],
                                xch[:, ko, j * P : (j + 1) * P],
                                wv_sb[:, ko, :],
                                start=(ko == 0),
                                stop=(ko == KO - 1),
                            )
                        nc.scalar.copy(v_sb[:, kt_idx, :], psv[:, :DV])

            # ---- Phase B: attention (transposed scores, 2 heads packed) ----
            CH = 2  # key tiles per psum score chunk
            with (
                tc.tile_pool(name="psS", bufs=4, space="PSUM") as psS,
                tc.tile_pool(name="psO", bufs=1, space="PSUM") as psO,
                tc.tile_pool(name="psL", bufs=2, space="PSUM") as psL,
                tc.tile_pool(name="psR", bufs=1, space="PSUM") as psR,
            ):
                # per-qt state carried one iteration (software pipeline)
                pend = None  # (tiles, e_sb, ps_l, qt)

                def emit_tail(pend):
                    tiles, e_sb, ps_l, qt = pend
                    nk = len(tiles)
                    # 1/l, broadcast to all partitions via rank-1 matmul
                    rl = lpool.tile([1, 2 * P], F32, tag="rl")
                    nc.vector.reciprocal(rl[:], ps_l[0:1, :])
                    ps_r = psR.tile([P, 2 * P], F32, tag="psR")
                    nc.tensor.matmul(
                        ps_r[:], ones1[:], rl[:], start=True, stop=True
                    )
                    rlb = rbpool.tile([P, 2 * P], F32, tag="rlb")
                    nc.scalar.copy(rlb[:], ps_r[:])
                    # PV accumulate: outT_u[dv, (h,q)] = sum_kt v[kt]^T exp
                    ps_o = psO.tile([P, 2 * P], F32, tag="psO")
                    for j, (kt, mk) in enumerate(tiles):
                        nc.tensor.matmul(
                            ps_o[:],
                            v_sb[:, kt, :],
                            e_sb[:, j],
                            start=(j == 0),
                            stop=(j == nk - 1),
                        )
                    # normalize + store
                    nc.vector.tensor_tensor(
                        outT[:, :, qt, :],
                        ps_o[:].rearrange("p (h q) -> p h q", h=2),
                        rlb[:].rearrange("p (h q) -> p h q", h=2),
                        op=mybir.AluOpType.mult,
                    )

                for qt in range(NQT):
                    tiles = _key_tiles(qt)
                    nk = len(tiles)
                    e_sb = epool.tile([P, 7, 2, P], BF16, tag="e")
                    # scores in chunks of CH key tiles
                    for c0 in range(0, nk, CH):
                        cw = min(CH, nk - c0)
                        ps_s = psS.tile([P, CH, 2, P], F32, tag="psS")
                        for j in range(cw):
                            kt, mk = tiles[c0 + j]
                            klo = kt * P
                            for h in range(2):
                                nc.tensor.matmul(
                                    ps_s[:, j, h, :],
                                    kT[:, klo : klo + P],
                                    qT[:, h, qt * P : (qt + 1) * P],
                                    start=True,
                                    stop=True,
                                )
                            if mk != "full":
                                msk = mask_c if mk == "causal" else mask_a
                                nc.vector.tensor_tensor(
                                    ps_s[:, j],
                                    ps_s[:, j],
                                    msk[:],
                                    op=mybir.AluOpType.add,
                                )
                        nc.scalar.activation(
                            e_sb[:, c0 : c0 + cw],
                            ps_s[:, :cw],
                            mybir.ActivationFunctionType.Exp,
                            scale=inv_sqrt_dqk,
                        )
                    # l[q] = sum_k exp via ones-column matmuls
                    ps_l = psL.tile([1, 2 * P], F32, tag="psL")
                    for j in range(nk):
                        nc.tensor.matmul(
                            ps_l[:],
                            onesc[:],
                            e_sb[:, j],
                            start=(j == 0),
                            stop=(j == nk - 1),
                        )
                    # emit previous iteration's PV/normalize now that this
                    # iteration's PE work is queued (hides ACT/DVE latency)
                    if pend is not None:
                        emit_tail(pend)
                    pend = (tiles, e_sb, ps_l, qt)
                emit_tail(pend)

            # ---- Phase C: y = outT.T @ wo ----
            with tc.tile_pool(name="psY", bufs=2, space="PSUM") as psY:
                for st_i in range(NQT):
                    ys = ypool.tile([P, D], F32, tag="y")
                    for nn in range(4):
                        ps_y = psY.tile([P, SC], F32, tag="psY")
                        for h in range(2):
                            nc.tensor.matmul(
                                ps_y[:],
                                outT[:, h, st_i, :],
                                wo_sb[:, h, nn * SC : (nn + 1) * SC],
                                start=(h == 0),
                                stop=(h == 1),
                            )
                        if nn % 2 == 0:
                            nc.scalar.copy(ys[:, nn * SC : (nn + 1) * SC], ps_y[:])
                        else:
                            nc.vector.tensor_scalar_mul(
                                ys[:, nn * SC : (nn + 1) * SC], ps_y[:], 1.0
                            )
                    nc.sync.dma_start(y_d[st_i * P : (st_i + 1) * P, :], ys[:])

    return nc


_PROGRAM = None


def _get_program():
    global _PROGRAM
    if _PROGRAM is None:
        _PROGRAM = build_program()
        # Bacc legalization (register alloc, 1-wait-per-instruction split)
        # must run before serialization; the prebuilt-nc PJRT path does not
        # call finalize itself.
        _PROGRAM.finalize()
    return _PROGRAM


def _host_inputs(x, wq, wk, wv, wo):
    """Build per-core input maps (host-side sharding + preprocessing)."""
    x2 = np.asarray(x).reshape(S, D).astype(np.float32)
    xt_bf = x2.T.astype(ml_bf16)

    # paired RoPE basis permutation within each head
    perm = np.concatenate([np.arange(0, DQK, 2), np.arange(1, DQK, 2)])
    wq_p = np.asarray(wq).reshape(D, NH, DQK)[:, :, perm]
    wk_p = np.asarray(wk).reshape(D, NKV, DQK)[:, :, perm]
    wv_r = np.asarray(wv).reshape(D, NKV, DV)
    wo_r = np.asarray(wo).reshape(NH, DV, D)

    inv_freq = 1.0 / (THETA ** (np.arange(0, DQK, 2)[: DQK // 2] / DQK))
    t = np.arange(S, dtype=np.float64)
    ang = np.outer(t, inv_freq)  # (S, 64)
    cos_half = np.cos(ang).T.astype(np.float32)  # (64, S)
    sin_half = np.sin(ang).T.astype(np.float32)
    cos_dup = np.concatenate([cos_half, cos_half], 0)  # (128, S)
    sin_dup = np.concatenate([sin_half, sin_half], 0)

    # transposed-tile masks [k, q], duplicated for the packed 2-head layout
    a = np.arange(P)[:, None]  # k (partition)
    b = np.arange(P)[None, :]  # q (free)
    mct = np.where(a <= b, 0.0, NEG).astype(np.float32)  # causal: k <= q
    mat = np.where(a > b, 0.0, NEG).astype(np.float32)  # anti: k > q
    mask_cT = np.concatenate([mct, mct], axis=1)  # (128, 256)
    mask_aT = np.concatenate([mat, mat], axis=1)
    onesc = np.ones((P, 1), dtype=np.float32).astype(ml_bf16)
    ones1 = np.ones((1, P), dtype=np.float32)

    in_maps = []
    for c in range(NC_):
        kvh = c // 2
        in_maps.append(
            {
                "xt": xt_bf,
                "wq": np.ascontiguousarray(
                    wq_p[:, 2 * c : 2 * c + 2, :].reshape(D, 2 * DQK)
                ).astype(ml_bf16),
                "wk": np.ascontiguousarray(wk_p[:, kvh, :]).astype(ml_bf16),
                "wv": np.ascontiguousarray(wv_r[:, kvh, :]).astype(ml_bf16),
                "wo": np.ascontiguousarray(
                    wo_r[2 * c : 2 * c + 2].reshape(2 * DV, D)
                ).astype(ml_bf16),
                "cosd": cos_dup,
                "sind": sin_dup,
                "maskcT": mask_cT,
                "maskaT": mask_aT,
                "onesc": onesc,
                "ones1": ones1,
            }
        )
    return in_maps


try:
    import ml_dtypes

    ml_bf16 = ml_dtypes.bfloat16
except ImportError:  # pragma: no cover
    ml_bf16 = np.float32


def kernel(x, wq, wk, wv, wo, _trace=False, _trace_kwargs=None):
    nc = _get_program()
    in_maps = _host_inputs(x, wq, wk, wv, wo)
    res = run_bass_kernel_spmd(
        nc, in_maps, list(range(NC_)), trace=_trace, **(_trace_kwargs or {})
    )
    y = np.zeros((S, D), np.float32)
    for r in res.results:
        y += np.asarray(r["y"], np.float32)
    out = y.reshape(1, S, D)
    if _trace:
        return out, res
    return out


# revision 6
# speedup vs baseline: 1.6156x; 1.2246x over previous
"""Sparse-attention (sliding window 512 + front 256) Trainium2 kernel.

Head-sharded across 8 NeuronCores: core c computes q-heads {2c, 2c+1} and
kv-head c//2, producing a partial output y_c = attn_out_c @ wo_c; the host
sums the 8 partials.

Layout choices:
  - q/k projections are computed transposed (qT: [dqk, seq]) directly from a
    host-provided x^T, so score matmuls need no transposes. v is computed
    transposed the same way (wv stationary, N=512) then flipped to natural
    [seq, dv] tiles with PE transposes (4 per 512-chunk).
  - RoPE uses a "paired" head-dim basis (reals in dims 0..63, imags 64..127),
    obtained by permuting wq/wk columns on the host. Dot products are
    permutation-invariant, so scores are unchanged.
  - Scores are computed TRANSPOSED (st[k, q] via lhsT=k-tile, rhs=q-tile), so
    the PV matmul (lhsT=v-tile, rhs=exp-tile) directly yields outT [dv, q]
    with no 128x128 transposes of the probability matrix.
  - Both q-heads of the core share the kv head, so exp/l/PV run on packed
    [k, 2*128] tiles (h0|h1 side by side).
  - Softmax denominators l[q] = sum_k exp come from a ones-column matmul
    accumulated over key tiles; 1/l (reciprocal_approx_fast) is broadcast to
    all partitions with a rank-1 outer-product matmul.
  - The output projection for query tile qt is emitted right after qt's
    attention tail, so its matmuls/copies/DMA pipeline with attention instead
    of forming a serial output phase. y is stored bf16 (error budget 2e-2).
  - Sparse mask: per 128x128 tile the mask is full, causal or anti-causal;
    the two nontrivial patterns are passed as constants (transposed, and
    duplicated for the packed two-head layout).
  - Softmax without max-subtraction (scores ~ N(0,1) after 1/sqrt(128) scale,
    so exp() is safe in fp32).
"""

import math
import sys

import numpy as np

sys.path.insert(0, "/opt/trn_rl_repo")

import concourse.bass as bass
from concourse import bacc
import concourse.mybir as mybir
import concourse.tile as tile
from concourse.bass_utils import run_bass_kernel_spmd

# Problem constants (hardcoded per contract)
S = 4096
D = 2048
NH = 16
NKV = 4
DQK = 128
DV = 128
WIN = 512
FRONT = 256
THETA = 10000.0
P = 128
NQT = S // P  # 32 query tiles
NC_ = 8  # cores
SC = 512  # seq chunk for projections
NSC = S // SC  # 8
KO = D // P  # 16 contraction chunks

F32 = mybir.dt.float32
BF16 = mybir.dt.bfloat16

NEG = -1.0e9


def _key_tiles(qt):
    """Key tiles for query tile qt: list of (kt, mask) with mask in
    {'full','causal','anti'}."""
    if qt <= 5:
        tiles = [(kt, "full") for kt in range(qt)] + [(qt, "causal")]
    else:
        tiles = [(0, "full"), (1, "full"), (qt - 4, "anti")]
        tiles += [(kt, "full") for kt in range(qt - 3, qt)]
        tiles += [(qt, "causal")]
    return tiles


def build_program():
    nc = bacc.Bacc(None, target_bir_lowering=False)

    xt = nc.dram_tensor("xt", [D, S], BF16, kind="ExternalInput")
    wq_d = nc.dram_tensor("wq", [D, 2 * DQK], BF16, kind="ExternalInput")
    wk_d = nc.dram_tensor("wk", [D, DQK], BF16, kind="ExternalInput")
    wv_d = nc.dram_tensor("wv", [D, DV], BF16, kind="ExternalInput")
    wo_d = nc.dram_tensor("wo", [2 * DV, D], BF16, kind="ExternalInput")
    cos_d = nc.dram_tensor("cosd", [P, S], F32, kind="ExternalInput")
    sin_d = nc.dram_tensor("sind", [P, S], F32, kind="ExternalInput")
    mask_c_d = nc.dram_tensor("maskcT", [P, 2 * P], F32, kind="ExternalInput")
    mask_a_d = nc.dram_tensor("maskaT", [P, 2 * P], F32, kind="ExternalInput")
    onesc_d = nc.dram_tensor("onesc", [P, 1], BF16, kind="ExternalInput")
    ones1_d = nc.dram_tensor("ones1", [1, P], F32, kind="ExternalInput")
    ident_d = nc.dram_tensor("ident", [P, P], BF16, kind="ExternalInput")
    y_d = nc.dram_tensor("y", [S, D], BF16, kind="ExternalOutput")

    inv_sqrt_dqk = 1.0 / math.sqrt(DQK)

    with tile.TileContext(nc) as tc:
        with (
            tc.tile_pool(name="persist", bufs=1) as persist,
            tc.tile_pool(name="xchunk", bufs=2) as xpool,
            tc.tile_pool(name="stage", bufs=3) as stage,
            tc.tile_pool(name="epool", bufs=2) as epool,
            tc.tile_pool(name="lpool", bufs=2) as lpool,
            tc.tile_pool(name="rbpool", bufs=2) as rbpool,
            tc.tile_pool(name="ystage", bufs=2) as ypool,
        ):
            # ---- persistent SBUF tensors ----
            qT = persist.tile([P, 2, S], BF16, tag="qT")
            kT = persist.tile([P, S], BF16, tag="kT")
            v_sb = persist.tile([P, NQT, DV], BF16, tag="v")
            outT = persist.tile([P, 2, NQT, P], BF16, tag="outT")
            cos_sb = persist.tile([P, S], F32, tag="cos")
            sin_sb = persist.tile([P, S], F32, tag="sin")
            wq_sb = persist.tile([P, KO, 2 * DQK], BF16, tag="wq")
            wk_sb = persist.tile([P, KO, DQK], BF16, tag="wk")
            wv_sb = persist.tile([P, KO, DV], BF16, tag="wv")
            wo_sb = persist.tile([P, 2, D], BF16, tag="wo")
            mask_c = persist.tile([P, 2, P], F32, tag="mc")
            mask_a = persist.tile([P, 2, P], F32, tag="ma")
            onesc = persist.tile([P, 1], BF16, tag="onesc")
            ones1 = persist.tile([1, P], F32, tag="ones1")
            ident = persist.tile([P, P], BF16, tag="id")

            # weights needed by the first projection matmuls go first; the
            # rest is issued after the first x chunk so it overlaps compute
            nc.sync.dma_start(wq_sb[:], wq_d.rearrange("(ko p) m -> p ko m", p=P))
            nc.sync.dma_start(wk_sb[:], wk_d.rearrange("(ko p) m -> p ko m", p=P))
            nc.sync.dma_start(wv_sb[:], wv_d.rearrange("(ko p) m -> p ko m", p=P))

            xt_r = xt.rearrange("(ko p) s -> p ko s", p=P)

            # ---- Phase A: projections + RoPE ----
            with (
                tc.tile_pool(name="psA", bufs=2, space="PSUM") as psA,
                tc.tile_pool(name="psT", bufs=2, space="PSUM") as psT,
            ):
                for sc in range(NSC):
                    ssl = slice(sc * SC, (sc + 1) * SC)
                    xch = xpool.tile([P, KO, SC], BF16, tag="xch")
                    nc.sync.dma_start(xch[:], xt_r[:, :, ssl])

                    if sc == 0:
                        # late bulk constants (not needed for the first MMs)
                        nc.sync.dma_start(cos_sb[:], cos_d[:])
                        nc.sync.dma_start(sin_sb[:], sin_d[:])
                        nc.sync.dma_start(ident[:], ident_d[:])
                        nc.sync.dma_start(onesc[:], onesc_d[:])
                        nc.sync.dma_start(ones1[:], ones1_d[:])
                        nc.sync.dma_start(
                            mask_c[:], mask_c_d.rearrange("p (h q) -> p h q", h=2)
                        )
                        nc.sync.dma_start(
                            mask_a[:], mask_a_d.rearrange("p (h q) -> p h q", h=2)
                        )
                        nc.sync.dma_start(
                            wo_sb[:], wo_d.rearrange("(h p) n -> p h n", p=P)
                        )

                    # qT (2 head tiles), kT (RoPE'd) and vT
                    for m in range(4):
                        ps = psA.tile([P, SC], F32, tag="psA")
                        if m < 2:
                            w_ap = wq_sb[:, :, m * P : (m + 1) * P]
                        elif m == 2:
                            w_ap = wk_sb
                        else:
                            w_ap = wv_sb
                        for ko in range(KO):
                            nc.tensor.matmul(
                                ps[:],
                                w_ap[:, ko, :],
                                xch[:, ko, :],
                                start=(ko == 0),
                                stop=(ko == KO - 1),
                            )
                        if m == 3:
                            # vT chunk -> natural v tiles via PE transposes
                            vt_st = stage.tile([P, SC], BF16, tag="vts")
                            nc.scalar.copy(vt_st[:], ps[:])
                            for j in range(SC // P):
                                pst = psT.tile([P, P], BF16, tag="psT")
                                nc.tensor.transpose(
                                    pst[:], vt_st[:, j * P : (j + 1) * P], ident[:]
                                )
                                nc.scalar.copy(
                                    v_sb[:, sc * (SC // P) + j, :], pst[:]
                                )
                            continue
                        dst = qT[:, m, ssl] if m < 2 else kT[:, ssl]
                        st = stage.tile([P, SC], F32, tag="ropestage")
                        nc.scalar.copy(st[:], ps[:])
                        # RoPE (paired layout): rows 0:64 real, 64:128 imag
                        trc = stage.tile([P, SC], F32, tag="trc")
                        trs = stage.tile([P, SC], F32, tag="trs")
                        nc.vector.tensor_tensor(
                            trc[:], st[:], cos_sb[:, ssl], op=mybir.AluOpType.mult
                        )
                        nc.vector.tensor_tensor(
                            trs[:], st[:], sin_sb[:, ssl], op=mybir.AluOpType.mult
                        )
                        # out_r = r*c - i*s ; out_i = r*s + i*c
                        # (DVE needs equal base partitions for both SBUF inputs,
                        #  so stage the upper halves at partition 0 first)
                        his = stage.tile([64, SC], F32, tag="his")
                        hic = stage.tile([64, SC], F32, tag="hic")
                        nc.scalar.copy(his[:], trs[64:128])
                        nc.scalar.copy(hic[:], trc[64:128])
                        nc.vector.tensor_tensor(
                            dst[0:64], trc[0:64], his[:], op=mybir.AluOpType.subtract
                        )
                        nc.vector.tensor_tensor(
                            dst[64:128], trs[0:64], hic[:], op=mybir.AluOpType.add
                        )

            # ---- Phase B: attention + fused output projection ----
            CH = 2  # key tiles per psum score chunk
            with (
                tc.tile_pool(name="psS", bufs=3, space="PSUM") as psS,
                tc.tile_pool(name="psO", bufs=1, space="PSUM") as psO,
                tc.tile_pool(name="psL", bufs=1, space="PSUM") as psL,
                tc.tile_pool(name="psR", bufs=1, space="PSUM") as psR,
                tc.tile_pool(name="psY", bufs=2, space="PSUM") as psY,
            ):
                pend = None  # (tiles, e_sb, ps_l, qt) pipelined one iteration
                rl_pend = None

                def emit_recip(pend):
                    # 1/l on the single-partition row (cheap custom DVE op)
                    _, _, ps_l, _ = pend
                    rl = lpool.tile([1, 2 * P], F32, tag="rl")
                    nc.vector.reciprocal_approx_fast(rl[:], ps_l[0:1, :])
                    return rl

                def emit_tail(pend, rl):
                    tiles, e_sb, _, qt = pend
                    nk = len(tiles)
                    # broadcast 1/l to all partitions via rank-1 matmul
                    ps_r = psR.tile([P, 2 * P], F32, tag="psR")
                    nc.tensor.matmul(
                        ps_r[:], ones1[:], rl[:], start=True, stop=True
                    )
                    rlb = rbpool.tile([P, 2 * P], F32, tag="rlb")
                    nc.scalar.copy(rlb[:], ps_r[:])
                    # PV accumulate: outT_u[dv, (h,q)] = sum_kt v[kt]^T exp
                    ps_o = psO.tile([P, 2 * P], F32, tag="psO")
                    for j, (kt, mk) in enumerate(tiles):
                        nc.tensor.matmul(
                            ps_o[:],
                            v_sb[:, kt, :],
                            e_sb[:, j],
                            start=(j == 0),
                            stop=(j == nk - 1),
                        )
                    # normalize + store
                    nc.vector.tensor_tensor(
                        outT[:, :, qt, :],
                        ps_o[:].rearrange("p (h q) -> p h q", h=2),
                        rlb[:].rearrange("p (h q) -> p h q", h=2),
                        op=mybir.AluOpType.mult,
                    )
                    # output projection for this query tile (pipelines with
                    # attention instead of forming a serial phase)
                    ys = ypool.tile([P, D], BF16, tag="y")
                    for nn in range(4):
                        ps_y = psY.tile([P, SC], F32, tag="psY")
                        for h in range(2):
                            nc.tensor.matmul(
                                ps_y[:],
                                outT[:, h, qt, :],
                                wo_sb[:, h, nn * SC : (nn + 1) * SC],
                                start=(h == 0),
                                stop=(h == 1),
                            )
                        if nn % 2 == 0:
                            nc.scalar.copy(ys[:, nn * SC : (nn + 1) * SC], ps_y[:])
                        else:
                            nc.vector.tensor_scalar_mul(
                                ys[:, nn * SC : (nn + 1) * SC], ps_y[:], 1.0
                            )
                    nc.sync.dma_start(y_d[qt * P : (qt + 1) * P, :], ys[:])

                for qt in range(NQT):
                    if pend is not None:
                        rl_pend = emit_recip(pend)
                    tiles = _key_tiles(qt)
                    nk = len(tiles)
                    e_sb = epool.tile([P, 7, 2, P], BF16, tag="e")
                    # scores in chunks of CH key tiles
                    for c0 in range(0, nk, CH):
                        cw = min(CH, nk - c0)
                        ps_s = psS.tile([P, CH, 2, P], F32, tag="psS")
                        for j in range(cw):
                            kt, mk = tiles[c0 + j]
                            klo = kt * P
                            for h in range(2):
                                nc.tensor.matmul(
                                    ps_s[:, j, h, :],
                                    kT[:, klo : klo + P],
                                    qT[:, h, qt * P : (qt + 1) * P],
                                    start=True,
                                    stop=True,
                                )
                            if mk != "full":
                                msk = mask_c if mk == "causal" else mask_a
                                nc.vector.tensor_tensor(
                                    ps_s[:, j],
                                    ps_s[:, j],
                                    msk[:],
                                    op=mybir.AluOpType.add,
                                )
                        nc.scalar.activation(
                            e_sb[:, c0 : c0 + cw],
                            ps_s[:, :cw],
                            mybir.ActivationFunctionType.Exp,
                            scale=inv_sqrt_dqk,
                        )
                    # previous tile's PV / normalize / output projection
                    if pend is not None:
                        emit_tail(pend, rl_pend)
                    # l[q] = sum_k exp via ones-column matmuls
                    ps_l = psL.tile([1, 2 * P], F32, tag="psL")
                    for j in range(nk):
                        nc.tensor.matmul(
                            ps_l[:],
                            onesc[:],
                            e_sb[:, j],
                            start=(j == 0),
                            stop=(j == nk - 1),
                        )
                    pend = (tiles, e_sb, ps_l, qt)
                rl_pend = emit_recip(pend)
                emit_tail(pend, rl_pend)

    return nc


_PROGRAM = None


def _get_program():
    global _PROGRAM
    if _PROGRAM is None:
        _PROGRAM = build_program()
        # Bacc legalization (register alloc, 1-wait-per-instruction split)
        # must run before serialization; the prebuilt-nc PJRT path does not
        # call finalize itself.
        _PROGRAM.finalize()
    return _PROGRAM


def _host_inputs(x, wq, wk, wv, wo):
    """Build per-core input maps (host-side sharding + preprocessing)."""
    x2 = np.asarray(x).reshape(S, D).astype(np.float32)
    xt_bf = x2.T.astype(ml_bf16)

    # paired RoPE basis permutation within each head
    perm = np.concatenate([np.arange(0, DQK, 2), np.arange(1, DQK, 2)])
    wq_p = np.asarray(wq).reshape(D, NH, DQK)[:, :, perm]
    wk_p = np.asarray(wk).reshape(D, NKV, DQK)[:, :, perm]
    wv_r = np.asarray(wv).reshape(D, NKV, DV)
    wo_r = np.asarray(wo).reshape(NH, DV, D)

    inv_freq = 1.0 / (THETA ** (np.arange(0, DQK, 2)[: DQK // 2] / DQK))
    t = np.arange(S, dtype=np.float64)
    ang = np.outer(t, inv_freq)  # (S, 64)
    cos_half = np.cos(ang).T.astype(np.float32)  # (64, S)
    sin_half = np.sin(ang).T.astype(np.float32)
    cos_dup = np.concatenate([cos_half, cos_half], 0)  # (128, S)
    sin_dup = np.concatenate([sin_half, sin_half], 0)

    # transposed-tile masks [k, q], duplicated for the packed 2-head layout
    a = np.arange(P)[:, None]  # k (partition)
    b = np.arange(P)[None, :]  # q (free)
    mct = np.where(a <= b, 0.0, NEG).astype(np.float32)  # causal: k <= q
    mat = np.where(a > b, 0.0, NEG).astype(np.float32)  # anti: k > q
    mask_cT = np.concatenate([mct, mct], axis=1)  # (128, 256)
    mask_aT = np.concatenate([mat, mat], axis=1)
    onesc = np.ones((P, 1), dtype=np.float32).astype(ml_bf16)
    ones1 = np.ones((1, P), dtype=np.float32)
    ident = np.eye(P, dtype=np.float32).astype(ml_bf16)

    in_maps = []
    for c in range(NC_):
        kvh = c // 2
        in_maps.append(
            {
                "xt": xt_bf,
                "wq": np.ascontiguousarray(
                    wq_p[:, 2 * c : 2 * c + 2, :].reshape(D, 2 * DQK)
                ).astype(ml_bf16),
                "wk": np.ascontiguousarray(wk_p[:, kvh, :]).astype(ml_bf16),
                "wv": np.ascontiguousarray(wv_r[:, kvh, :]).astype(ml_bf16),
                "wo": np.ascontiguousarray(
                    wo_r[2 * c : 2 * c + 2].reshape(2 * DV, D)
                ).astype(ml_bf16),
                "cosd": cos_dup,
                "sind": sin_dup,
                "maskcT": mask_cT,
                "maskaT": mask_aT,
                "onesc": onesc,
                "ones1": ones1,
                "ident": ident,
            }
        )
    return in_maps


try:
    import ml_dtypes

    ml_bf16 = ml_dtypes.bfloat16
except ImportError:  # pragma: no cover
    ml_bf16 = np.float32


def kernel(x, wq, wk, wv, wo, _trace=False, _trace_kwargs=None):
    nc = _get_program()
    in_maps = _host_inputs(x, wq, wk, wv, wo)
    res = run_bass_kernel_spmd(
        nc, in_maps, list(range(NC_)), trace=_trace, **(_trace_kwargs or {})
    )
    y = np.zeros((S, D), np.float32)
    for r in res.results:
        y += np.asarray(r["y"], np.float32)
    out = y.reshape(1, S, D)
    if _trace:
        return out, res
    return out


# revision 22
# speedup vs baseline: 1.8558x; 1.1487x over previous
"""Sparse-attention (sliding window 512 + front 256) Trainium2 kernel.

Head-sharded across 8 NeuronCores: core c computes q-heads {2c, 2c+1} and
kv-head c//2, producing a partial output y_c = attn_out_c @ wo_c; the host
sums the 8 partials.

Layout choices:
  - q/k projections are computed transposed (qT: [dqk, seq]) directly from a
    host-provided x^T, so score matmuls need no transposes. v is computed
    transposed the same way (wv stationary, N=512) then flipped to natural
    [seq, dv] tiles with PE transposes (4 per 512-chunk).
  - RoPE uses a "paired" head-dim basis (reals in dims 0..63, imags 64..127),
    obtained by permuting wq/wk columns on the host. Dot products are
    permutation-invariant, so scores are unchanged.
  - Scores are computed TRANSPOSED (st[k, q] via lhsT=k-tile, rhs=q-tile), so
    the PV matmul (lhsT=v-tile, rhs=exp-tile) directly yields outT [dv, q]
    with no 128x128 transposes of the probability matrix.
  - Both q-heads of the core share the kv head, so exp/l/PV run on packed
    [k, 2*128] tiles (h0|h1 side by side).
  - Softmax denominators l[q] = sum_k exp come from a ones-column matmul
    accumulated over key tiles; 1/l (reciprocal_approx_fast) is broadcast to
    all partitions with a rank-1 outer-product matmul.
  - The output projection for query tile qt is emitted right after qt's
    attention tail, so its matmuls/copies/DMA pipeline with attention instead
    of forming a serial output phase. y is stored bf16 (error budget 2e-2).
  - Sparse mask: per 128x128 tile the mask is full, causal or anti-causal;
    the two nontrivial patterns are passed as constants (transposed, and
    duplicated for the packed two-head layout).
  - Softmax without max-subtraction (scores ~ N(0,1) after 1/sqrt(128) scale,
    so exp() is safe in fp32).
"""

import math
import sys

import numpy as np

sys.path.insert(0, "/opt/trn_rl_repo")

import concourse.bass as bass
from concourse import bacc
import concourse.mybir as mybir
import concourse.tile as tile
from concourse.bass_utils import run_bass_kernel_spmd

# Problem constants (hardcoded per contract)
S = 4096
D = 2048
NH = 16
NKV = 4
DQK = 128
DV = 128
WIN = 512
FRONT = 256
THETA = 10000.0
P = 128
NQT = S // P  # 32 query tiles
NC_ = 8  # cores
SC = 512  # seq chunk for projections
NSC = S // SC  # 8
KO = D // P  # 16 contraction chunks

F32 = mybir.dt.float32
BF16 = mybir.dt.bfloat16

NEG = -1.0e9


def _key_tiles(qt):
    """Key tiles for query tile qt: list of (kt, mask) with mask in
    {'full','causal','anti'}."""
    if qt <= 5:
        tiles = [(kt, "full") for kt in range(qt)] + [(qt, "causal")]
    else:
        tiles = [(0, "full"), (1, "full"), (qt - 4, "anti")]
        tiles += [(kt, "full") for kt in range(qt - 3, qt)]
        tiles += [(qt, "causal")]
    return tiles


def _pair_tiles(q0):
    """Union key tiles for the query-tile pair (q0, q0+1) with per-half mask
    patterns: (kt, p0, p1), pX in {'full','causal','anti','dead'}."""
    t0 = dict(_key_tiles(q0))
    t1 = dict(_key_tiles(q0 + 1))
    return [
        (kt, t0.get(kt, "dead"), t1.get(kt, "dead"))
        for kt in sorted(set(t0) | set(t1))
    ]


def build_program():
    nc = bacc.Bacc(None, target_bir_lowering=False)

    xt = nc.dram_tensor("xt", [D, S], BF16, kind="ExternalInput")
    wq_d = nc.dram_tensor("wq", [D, 2 * DQK], BF16, kind="ExternalInput")
    wk_d = nc.dram_tensor("wk", [D, DQK], BF16, kind="ExternalInput")
    wv_d = nc.dram_tensor("wv", [D, DV], BF16, kind="ExternalInput")
    wo_d = nc.dram_tensor("wo", [2 * DV, D], BF16, kind="ExternalInput")
    cos_d = nc.dram_tensor("cosd", [P, S], F32, kind="ExternalInput")
    sin_d = nc.dram_tensor("sind", [P, S], F32, kind="ExternalInput")
    mask_c_d = nc.dram_tensor("maskcT", [P, P], F32, kind="ExternalInput")
    mask_a_d = nc.dram_tensor("maskaT", [P, P], F32, kind="ExternalInput")
    onesc_d = nc.dram_tensor("onesc", [P, 1], BF16, kind="ExternalInput")
    ones1_d = nc.dram_tensor("ones1", [1, P], F32, kind="ExternalInput")
    ident_d = nc.dram_tensor("ident", [P, P], BF16, kind="ExternalInput")
    y_d = nc.dram_tensor("y", [S, D], BF16, kind="ExternalOutput")

    inv_sqrt_dqk = 1.0 / math.sqrt(DQK)

    with tile.TileContext(nc) as tc:
        with (
            tc.tile_pool(name="persist", bufs=1) as persist,
            tc.tile_pool(name="xchunk", bufs=2) as xpool,
            tc.tile_pool(name="stage", bufs=3) as stage,
            tc.tile_pool(name="epool", bufs=2) as epool,
            tc.tile_pool(name="lpool", bufs=2) as lpool,
            tc.tile_pool(name="rbpool", bufs=2) as rbpool,
            tc.tile_pool(name="ystage", bufs=2) as ypool,
        ):
            # ---- persistent SBUF tensors ----
            qT = persist.tile([P, 2, S], BF16, tag="qT")
            kT = persist.tile([P, S], BF16, tag="kT")
            v_sb = persist.tile([P, NQT, DV], BF16, tag="v")
            outT = persist.tile([P, 2, NQT, P], BF16, tag="outT")
            cos_sb = persist.tile([P, S], F32, tag="cos")
            sin_sb = persist.tile([P, S], F32, tag="sin")
            wq_sb = persist.tile([P, KO, 2 * DQK], BF16, tag="wq")
            wk_sb = persist.tile([P, KO, DQK], BF16, tag="wk")
            wv_sb = persist.tile([P, KO, DV], BF16, tag="wv")
            wo_sb = persist.tile([P, 2, D], BF16, tag="wo")
            mask_c = persist.tile([P, P], F32, tag="mc")
            mask_a = persist.tile([P, P], F32, tag="ma")
            onesc = persist.tile([P, 1], BF16, tag="onesc")
            ones1 = persist.tile([1, P], F32, tag="ones1")
            ident = persist.tile([P, P], BF16, tag="id")

            # weights needed by the first projection matmuls go first; the
            # rest is issued after the first x chunk so it overlaps compute
            nc.sync.dma_start(wq_sb[:], wq_d.rearrange("(ko p) m -> p ko m", p=P))
            nc.sync.dma_start(wk_sb[:], wk_d.rearrange("(ko p) m -> p ko m", p=P))
            nc.sync.dma_start(wv_sb[:], wv_d.rearrange("(ko p) m -> p ko m", p=P))

            xt_r = xt.rearrange("(ko p) s -> p ko s", p=P)

            # ---- Phase A: projections + RoPE ----
            with (
                tc.tile_pool(name="psA", bufs=2, space="PSUM") as psA,
                tc.tile_pool(name="psT", bufs=2, space="PSUM") as psT,
            ):
                for sc in range(NSC):
                    ssl = slice(sc * SC, (sc + 1) * SC)
                    xch = xpool.tile([P, KO, SC], BF16, tag="xch")
                    nc.sync.dma_start(xch[:], xt_r[:, :, ssl])

                    if sc == 0:
                        # late bulk constants (not needed for the first MMs)
                        nc.sync.dma_start(cos_sb[:], cos_d[:])
                        nc.sync.dma_start(sin_sb[:], sin_d[:])
                        nc.sync.dma_start(ident[:], ident_d[:])
                        nc.sync.dma_start(onesc[:], onesc_d[:])
                        nc.sync.dma_start(ones1[:], ones1_d[:])
                        nc.sync.dma_start(mask_c[:], mask_c_d[:])
                        nc.sync.dma_start(mask_a[:], mask_a_d[:])
                        nc.sync.dma_start(
                            wo_sb[:], wo_d.rearrange("(h p) n -> p h n", p=P)
                        )

                    # qT (2 head tiles), kT (RoPE'd) and vT
                    for m in range(4):
                        ps = psA.tile([P, SC], F32, tag="psA")
                        if m < 2:
                            w_ap = wq_sb[:, :, m * P : (m + 1) * P]
                        elif m == 2:
                            w_ap = wk_sb
                        else:
                            w_ap = wv_sb
                        for ko in range(KO):
                            nc.tensor.matmul(
                                ps[:],
                                w_ap[:, ko, :],
                                xch[:, ko, :],
                                start=(ko == 0),
                                stop=(ko == KO - 1),
                            )
                        if m == 3:
                            # vT chunk -> natural v tiles via PE transposes
                            vt_st = stage.tile([P, SC], BF16, tag="vts")
                            nc.scalar.copy(vt_st[:], ps[:])
                            for j in range(SC // P):
                                pst = psT.tile([P, P], BF16, tag="psT")
                                nc.tensor.transpose(
                                    pst[:], vt_st[:, j * P : (j + 1) * P], ident[:]
                                )
                                nc.scalar.copy(
                                    v_sb[:, sc * (SC // P) + j, :], pst[:]
                                )
                            continue
                        dst = qT[:, m, ssl] if m < 2 else kT[:, ssl]
                        st = stage.tile([P, SC], F32, tag="ropestage")
                        nc.scalar.copy(st[:], ps[:])
                        # RoPE (paired layout): rows 0:64 real, 64:128 imag
                        trc = stage.tile([P, SC], F32, tag="trc")
                        trs = stage.tile([P, SC], F32, tag="trs")
                        nc.vector.tensor_tensor(
                            trc[:], st[:], cos_sb[:, ssl], op=mybir.AluOpType.mult
                        )
                        nc.vector.tensor_tensor(
                            trs[:], st[:], sin_sb[:, ssl], op=mybir.AluOpType.mult
                        )
                        # out_r = r*c - i*s ; out_i = r*s + i*c
                        # (DVE needs equal base partitions for both SBUF inputs,
                        #  so stage the upper halves at partition 0 first)
                        his = stage.tile([64, SC], F32, tag="his")
                        hic = stage.tile([64, SC], F32, tag="hic")
                        nc.scalar.copy(his[:], trs[64:128])
                        nc.scalar.copy(hic[:], trc[64:128])
                        nc.vector.tensor_tensor(
                            dst[0:64], trc[0:64], his[:], op=mybir.AluOpType.subtract
                        )
                        nc.vector.tensor_tensor(
                            dst[64:128], trs[0:64], hic[:], op=mybir.AluOpType.add
                        )

            # ---- Phase B: attention + fused output projection ----
            # Query tiles are processed in PAIRS (q0, q0+1): the score/l/PV
            # matmuls run at N=512 over [h0q0|h0q1|h1q0|h1q1] packed columns
            # (one rhs AP qT[:, :, q0*128 : q0*128+256]), halving the
            # instruction count vs per-tile processing.
            W4 = 4 * P  # 512 packed columns per pair
            with (
                tc.tile_pool(name="psS", bufs=3, space="PSUM") as psS,
                tc.tile_pool(name="psO", bufs=1, space="PSUM") as psO,
                tc.tile_pool(name="psL", bufs=1, space="PSUM") as psL,
                tc.tile_pool(name="psR", bufs=1, space="PSUM") as psR,
                tc.tile_pool(name="psY", bufs=2, space="PSUM") as psY,
            ):
                pend = None  # (tiles, e_sb, ps_l, q0) pipelined one pair
                rl_pend = None

                def emit_recip(pend):
                    # 1/l on the single-partition row (cheap custom DVE op)
                    _, _, ps_l, _ = pend
                    rl = lpool.tile([1, W4], F32, tag="rl")
                    nc.vector.reciprocal_approx_fast(rl[:], ps_l[0:1, :])
                    return rl

                def emit_tail(pend, rl):
                    tiles, e_sb, _, q0 = pend
                    nk = len(tiles)
                    # broadcast 1/l to all partitions via rank-1 matmul
                    ps_r = psR.tile([P, W4], F32, tag="psR")
                    nc.tensor.matmul(
                        ps_r[:], ones1[:], rl[:], start=True, stop=True
                    )
                    rlb = rbpool.tile([P, W4], F32, tag="rlb")
                    nc.scalar.copy(rlb[:], ps_r[:])
                    # PV accumulate: outT_u[dv, (h,qq,q)] = sum_kt v[kt]^T exp
                    ps_o = psO.tile([P, W4], F32, tag="psO")
                    for j, (kt, p0, p1) in enumerate(tiles):
                        nc.tensor.matmul(
                            ps_o[:],
                            v_sb[:, kt, :],
                            e_sb[:, j],
                            start=(j == 0),
                            stop=(j == nk - 1),
                        )
                    # normalize + store both query tiles of the pair
                    nc.vector.tensor_tensor(
                        outT[:, :, q0 : q0 + 2, :],
                        ps_o[:].rearrange("p (h qq q) -> p h qq q", h=2, qq=2),
                        rlb[:].rearrange("p (h qq q) -> p h qq q", h=2, qq=2),
                        op=mybir.AluOpType.mult,
                    )
                    # output projection for the pair (pipelines with attention
                    # instead of forming a serial phase)
                    for qq in range(2):
                        qt = q0 + qq
                        ys = ypool.tile([P, D], BF16, tag="y")
                        for nn in range(4):
                            ps_y = psY.tile([P, SC], F32, tag="psY")
                            for h in range(2):
                                nc.tensor.matmul(
                                    ps_y[:],
                                    outT[:, h, qt, :],
                                    wo_sb[:, h, nn * SC : (nn + 1) * SC],
                                    start=(h == 0),
                                    stop=(h == 1),
                                )
                            if nn % 2 == 0:
                                nc.scalar.copy(
                                    ys[:, nn * SC : (nn + 1) * SC], ps_y[:]
                                )
                            else:
                                nc.vector.tensor_scalar_mul(
                                    ys[:, nn * SC : (nn + 1) * SC], ps_y[:], 1.0
                                )
                        nc.sync.dma_start(y_d[qt * P : (qt + 1) * P, :], ys[:])

                for q0 in range(0, NQT, 2):
                    if pend is not None:
                        rl_pend = emit_recip(pend)
                    tiles = _pair_tiles(q0)
                    nk = len(tiles)
                    qcols = qT[:, :, q0 * P : (q0 + 2) * P]  # [P, 2, 256]
                    e_sb = epool.tile([P, 8, 2, 2, P], BF16, tag="e")
                    # scores, one key tile (one psum bank) at a time
                    for j, (kt, p0, p1) in enumerate(tiles):
                        ps_s = psS.tile([P, 2, 2, P], F32, tag="psS")
                        klo = kt * P
                        nc.tensor.matmul(
                            ps_s[:],
                            kT[:, klo : klo + P],
                            qcols,
                            start=True,
                            stop=True,
                        )
                        for qq, pat in ((0, p0), (1, p1)):
                            if pat == "full":
                                continue
                            half = ps_s[:, :, qq, :]
                            if pat == "dead":
                                nc.vector.tensor_scalar_add(half, half, NEG)
                            else:
                                msk = mask_c if pat == "causal" else mask_a
                                nc.vector.tensor_tensor(
                                    half,
                                    half,
                                    msk[:, None, :].to_broadcast((P, 2, P)),
                                    op=mybir.AluOpType.add,
                                )
                        nc.scalar.activation(
                            e_sb[:, j],
                            ps_s[:],
                            mybir.ActivationFunctionType.Exp,
                            scale=inv_sqrt_dqk,
                        )
                    # previous pair's PV / normalize / output projection
                    if pend is not None:
                        emit_tail(pend, rl_pend)
                    # l[q] = sum_k exp via ones-column matmuls
                    ps_l = psL.tile([1, W4], F32, tag="psL")
                    for j in range(nk):
                        nc.tensor.matmul(
                            ps_l[:],
                            onesc[:],
                            e_sb[:, j],
                            start=(j == 0),
                            stop=(j == nk - 1),
                        )
                    pend = (tiles, e_sb, ps_l, q0)
                rl_pend = emit_recip(pend)
                emit_tail(pend, rl_pend)

    return nc


_PROGRAM = None


def _get_program():
    global _PROGRAM
    if _PROGRAM is None:
        _PROGRAM = build_program()
        # Bacc legalization (register alloc, 1-wait-per-instruction split)
        # must run before serialization; the prebuilt-nc PJRT path does not
        # call finalize itself.
        _PROGRAM.finalize()
    return _PROGRAM


def _host_inputs(x, wq, wk, wv, wo):
    """Build per-core input maps (host-side sharding + preprocessing)."""
    x2 = np.asarray(x).reshape(S, D).astype(np.float32)
    xt_bf = x2.T.astype(ml_bf16)

    # paired RoPE basis permutation within each head
    perm = np.concatenate([np.arange(0, DQK, 2), np.arange(1, DQK, 2)])
    wq_p = np.asarray(wq).reshape(D, NH, DQK)[:, :, perm]
    wk_p = np.asarray(wk).reshape(D, NKV, DQK)[:, :, perm]
    wv_r = np.asarray(wv).reshape(D, NKV, DV)
    wo_r = np.asarray(wo).reshape(NH, DV, D)

    inv_freq = 1.0 / (THETA ** (np.arange(0, DQK, 2)[: DQK // 2] / DQK))
    t = np.arange(S, dtype=np.float64)
    ang = np.outer(t, inv_freq)  # (S, 64)
    cos_half = np.cos(ang).T.astype(np.float32)  # (64, S)
    sin_half = np.sin(ang).T.astype(np.float32)
    cos_dup = np.concatenate([cos_half, cos_half], 0)  # (128, S)
    sin_dup = np.concatenate([sin_half, sin_half], 0)

    # transposed-tile masks [k, q]
    a = np.arange(P)[:, None]  # k (partition)
    b = np.arange(P)[None, :]  # q (free)
    mask_cT = np.where(a <= b, 0.0, NEG).astype(np.float32)  # causal: k <= q
    mask_aT = np.where(a > b, 0.0, NEG).astype(np.float32)  # anti: k > q
    onesc = np.ones((P, 1), dtype=np.float32).astype(ml_bf16)
    ones1 = np.ones((1, P), dtype=np.float32)
    ident = np.eye(P, dtype=np.float32).astype(ml_bf16)

    in_maps = []
    for c in range(NC_):
        kvh = c // 2
        in_maps.append(
            {
                "xt": xt_bf,
                "wq": np.ascontiguousarray(
                    wq_p[:, 2 * c : 2 * c + 2, :].reshape(D, 2 * DQK)
                ).astype(ml_bf16),
                "wk": np.ascontiguousarray(wk_p[:, kvh, :]).astype(ml_bf16),
                "wv": np.ascontiguousarray(wv_r[:, kvh, :]).astype(ml_bf16),
                "wo": np.ascontiguousarray(
                    wo_r[2 * c : 2 * c + 2].reshape(2 * DV, D)
                ).astype(ml_bf16),
                "cosd": cos_dup,
                "sind": sin_dup,
                "maskcT": mask_cT,
                "maskaT": mask_aT,
                "onesc": onesc,
                "ones1": ones1,
                "ident": ident,
            }
        )
    return in_maps


try:
    import ml_dtypes

    ml_bf16 = ml_dtypes.bfloat16
except ImportError:  # pragma: no cover
    ml_bf16 = np.float32


def kernel(x, wq, wk, wv, wo, _trace=False, _trace_kwargs=None):
    nc = _get_program()
    in_maps = _host_inputs(x, wq, wk, wv, wo)
    res = run_bass_kernel_spmd(
        nc, in_maps, list(range(NC_)), trace=_trace, **(_trace_kwargs or {})
    )
    y = np.zeros((S, D), np.float32)
    for r in res.results:
        y += np.asarray(r["y"], np.float32)
    out = y.reshape(1, S, D)
    if _trace:
        return out, res
    return out
